# revision 1
# baseline (speedup 1.0000x reference)
"""DBISSF quad selective-scan (Mamba-style) for Trainium2, 8 NeuronCores.

Problem: 4 selective scans over (B=4, L=16384, d=192, n=4) with shared/cross
projections, each followed by LayerNorm.

Sharding: 8 cores = (batch b, scan-pair). Cores 0-3 run the "cross" pair
(y_rgb, y_e) for b = core; cores 4-7 run the "shared" pair (y_sr, y_se) for
b = core-4. One SPMD program: each core computes two scans
  scan1 = (u_a, proj_A own-B/delta, cross-C from proj_B)
  scan2 = (u_c, proj_B own-B/delta, cross-C from proj_A)
Cross cores: proj_A/proj_B inputs are x_rgb/x_e themselves. Shared cores:
both proj inputs are x_share_rgb + x_share_e (projections computed twice,
identical) and C cross-read equals C_s, so the same program yields y_sr/y_se.

Device algorithm per core (channel-major scan, d=192 per scan packed with the
partner scan into 3x128-partition tile groups):
  dbl = xp_w @ p.T          (PE; moving = host-pretransposed p)
  z   = dt_w @ dbl[0:6] + dt_b  (PE, bias via augmented ones-row)
  delta = softplus(z) = ln(1+exp(z))   (ACT, exp+ln share one table set)
  dA_n = exp(A_n * delta) built as g, g^2=Sq(g), g^4=Sq(g^2), g^3=g*g^2
  dBu_n = delta*u*B_n       (B_n broadcast across partitions via DMA)
  h_n = tensor_tensor_scan(dA_n, dBu_n)  (DVE, chained across chunks)
  y  = sum_n h_n*C_n + D*u
  LN: mean/var via PE ones-matmuls on y and y^2; rstd = exp(-0.5*ln(var+eps));
  PE-transpose y to token-major; (y-mu)*rstd via tensor_scalar; DMA out.
LayerNorm affine (w,b) applied on host (w=1,b=0 in this model anyway).
"""
import sys

sys.path.insert(0, "/opt/trn_rl_repo")

import numpy as np
from contextlib import ExitStack

import concourse.bass as bass
import concourse.tile as tile
from concourse import bacc, mybir
from concourse.masks import make_identity

F32 = mybir.dt.float32
BF = mybir.dt.bfloat16
AF = mybir.ActivationFunctionType
ALU = mybir.AluOpType

B, L, D, NST, RNK = 4, 16384, 192, 4, 6
LC = 1024               # chunk length (free dim)
NCHUNK = L // LC
TB = LC // 128          # 128-token transpose blocks per chunk
LN_EPS = 1e-5

_PROGRAM_CACHE = {}


# --------------------------------------------------------------------------
# device program
# --------------------------------------------------------------------------
def build_program(int_pow_a: bool, length=L, reps=0):
    """Build + compile the SPMD program. int_pow_a: dA powers via squaring."""
    key = (int_pow_a, length, reps)
    if key in _PROGRAM_CACHE:
        return _PROGRAM_CACHE[key]

    nchunk = length // LC
    NS = LC // 512            # 512-wide PSUM sub-slices per chunk
    nc = bacc.Bacc("TRN2", target_bir_lowering=False, debug=False, num_devices=8)

    dr = {}
    for nm in ("uta", "utc"):
        dr[nm] = nc.dram_tensor(nm, [D, length], BF, kind="ExternalInput").ap()
    for nm in ("pta", "ptb"):
        dr[nm] = nc.dram_tensor(nm, [D, length], F32, kind="ExternalInput").ap()
    for nm in ("xpa", "xpb"):
        dr[nm] = nc.dram_tensor(nm, [D, RNK + 2 * NST], F32, kind="ExternalInput").ap()
    for nm in ("dwa", "dwb"):
        dr[nm] = nc.dram_tensor(nm, [RNK, D], F32, kind="ExternalInput").ap()
    dr["dtbp"] = nc.dram_tensor("dtbp", [3, 128], F32, kind="ExternalInput").ap()
    dr["ascale"] = nc.dram_tensor("ascale", [NST, 3, 128], F32, kind="ExternalInput").ap()
    dr["dpk"] = nc.dram_tensor("dpk", [3, 128], F32, kind="ExternalInput").ap()
    bcd = nc.dram_tensor("bcd", [nchunk, 2, 2 * NST, LC], BF).ap()
    oa = nc.dram_tensor("oa", [length, D], F32, kind="ExternalOutput").ap()
    ob = nc.dram_tensor("ob", [length, D], F32, kind="ExternalOutput").ap()
    outs = (oa, ob)

    with tile.TileContext(nc) as tc, ExitStack() as ctx:
        cst = ctx.enter_context(tc.tile_pool(name="cst", bufs=1))
        io = ctx.enter_context(tc.tile_pool(name="io", bufs=2))
        wk = ctx.enter_context(tc.tile_pool(name="wk", bufs=3))
        gp = ctx.enter_context(tc.tile_pool(name="gp", bufs=13))
        bc = ctx.enter_context(tc.tile_pool(name="bc", bufs=2))
        hp = ctx.enter_context(tc.tile_pool(name="hp", bufs=14))
        zp = ctx.enter_context(tc.tile_pool(name="zpool", bufs=4))
        stp = ctx.enter_context(tc.tile_pool(name="stp", bufs=8))
        car = ctx.enter_context(tc.tile_pool(name="car", bufs=1))
        pp = ctx.enter_context(tc.tile_pool(name="pp", bufs=2, space="PSUM"))
        zps = ctx.enter_context(tc.tile_pool(name="zps", bufs=1, space="PSUM"))
        tps = ctx.enter_context(tc.tile_pool(name="tps", bufs=2, space="PSUM"))

        # ---- constants ----
        ident = cst.tile([128, 128], F32)
        make_identity(nc, ident)
        asc = cst.tile([128, NST * 3], F32)  # [p, n*3+k]
        nc.sync.dma_start(asc, dr["ascale"].rearrange("n k p -> p (n k)"))
        dpk = cst.tile([128, 3], F32)
        nc.sync.dma_start(dpk, dr["dpk"].rearrange("k p -> p k"))
        ones = cst.tile([128, 1], F32)
        nc.vector.memset(ones, 1.0)
        epsc = cst.tile([128, 1], F32)
        nc.vector.memset(epsc, LN_EPS)
        xph = cst.tile([128, RNK + 2 * NST], F32, tag="xph")
        nc.sync.dma_start(xph[0:64, :], dr["xpa"][128:D, :])
        nc.sync.dma_start(xph[64:128, :], dr["xpb"][128:D, :])
        xps = {}
        for X, hs in (("a", xph[0:64, :]), ("b", xph[64:128, :])):
            t0 = cst.tile([128, RNK + 2 * NST], F32, tag=f"xp{X}0", name=f"xp{X}0")
            nc.sync.dma_start(t0, dr[f"xp{X}"][0:128, :])
            xps[X] = (t0, hs)
        dws = {}
        for X in ("a", "b"):
            t0 = cst.tile([RNK, 128], F32, tag=f"dw{X}0", name=f"dw{X}0")
            nc.sync.dma_start(t0, dr[f"dw{X}"][:, 0:128])
            t1 = cst.tile([RNK, 64], F32, tag=f"dw{X}1", name=f"dw{X}1")
            nc.sync.dma_start(t1, dr[f"dw{X}"][:, 128:D])
            dws[X] = (t0, t1)
        dtb = cst.tile([128, 3], F32)
        nc.sync.dma_start(dtb, dr["dtbp"].rearrange("k p -> p k"))

        carry = [[car.tile([128, 1], F32, tag=f"car{n}_{k}", name=f"car{n}_{k}")
                  for k in range(3)] for n in range(NST)]
        for n in range(NST):
            for k in range(3):
                nc.vector.memset(carry[n][k], 0.0)

        ACOL = {(n, k): asc[:, n * 3 + k: n * 3 + k + 1] for n in range(NST)
                for k in range(3)}

        def chunk_body(c):
            sl = bass.ts(c, LC)
            # ---- loads (channel-major) ----
            ua = io.tile([128, LC], BF, tag="ua")
            nc.sync.dma_start(ua, dr["uta"][0:128, sl])
            ucl = io.tile([128, LC], BF, tag="uc")
            nc.sync.dma_start(ucl, dr["utc"][0:128, sl])
            um = io.tile([128, LC], BF, tag="um")
            nc.sync.dma_start(um[0:64, :], dr["uta"][128:D, sl])
            nc.sync.dma_start(um[64:128, :], dr["utc"][128:D, sl])
            pa = io.tile([128, LC], F32, tag="pa")
            nc.sync.dma_start(pa, dr["pta"][0:128, sl])
            pb = io.tile([128, LC], F32, tag="pb")
            nc.sync.dma_start(pb, dr["ptb"][0:128, sl])
            pm = io.tile([128, LC], F32, tag="pm")
            nc.sync.dma_start(pm[0:64, :], dr["pta"][128:D, sl])
            nc.sync.dma_start(pm[64:128, :], dr["ptb"][128:D, sl])
            uk = (ua, ucl, um)

            # ---- projections: dbl = xp_w @ p -> [14, LC] (512-wide slices) ----
            dbl = {}
            for X, mv_lo, mv_hi in (("a", pa, pm[0:64, :]), ("b", pb, pm[64:128, :])):
                sb = wk.tile([RNK + 2 * NST, LC], F32, tag="dbl", name=f"dbl{X}")
                for j in range(NS):
                    fs = bass.ts(j, 512)
                    ps = pp.tile([RNK + 2 * NST, 512], F32, tag="dblps")
                    nc.tensor.matmul(ps, xps[X][0], mv_lo[:, fs], start=True, stop=False)
                    nc.tensor.matmul(ps, xps[X][1], mv_hi[:, fs], start=False, stop=True)
                    nc.vector.tensor_copy(sb[:, fs], ps)
                dbl[X] = sb
                nc.gpsimd.dma_start(
                    bcd[c, 0 if X == "a" else 1], sb[RNK:RNK + 2 * NST, :])
            # ---- delta ----
            dl = []
            for k in range(3):
                d = zp.tile([128, LC], F32, tag="dl", name=f"dl{k}")
                for j in range(NS):
                    fs = bass.ts(j, 512)
                    zc = zps.tile([128, 512], F32, tag=f"zz{k}", name=f"zz{k}")
                    if k == 0:
                        nc.tensor.matmul(zc, dws["a"][0], dbl["a"][0:RNK, fs],
                                         start=True, stop=True)
                    elif k == 1:
                        nc.tensor.matmul(zc, dws["b"][0], dbl["b"][0:RNK, fs],
                                         start=True, stop=True)
                    else:
                        nc.tensor.matmul(zc[0:64, :], dws["a"][1], dbl["a"][0:RNK, fs],
                                         start=True, stop=True)
                        nc.tensor.matmul(zc[64:128, :], dws["b"][1], dbl["b"][0:RNK, fs],
                                         start=True, stop=True)
                    e = wk.tile([128, 512], F32, tag="esp", bufs=2)
                    nc.scalar.activation(e, zc, AF.Exp, bias=dtb[:, k: k + 1])
                    nc.scalar.activation(d[:, fs], e, AF.Ln, bias=1.0)
                dl.append(d)

            # ---- dA powers (bf16) ----
            g = [[None] * 3 for _ in range(NST)]
            for k in range(3):
                g1 = gp.tile([128, LC], F32, tag="g")
                nc.scalar.activation(g1, dl[k], AF.Exp, scale=ACOL[(0, k)])
                g[0][k] = g1
                if int_pow_a:
                    g2 = gp.tile([128, LC], F32, tag="g")
                    nc.scalar.activation(g2, g1, AF.Square)
                    g4 = gp.tile([128, LC], F32, tag="g")
                    nc.scalar.activation(g4, g2, AF.Square)
                    g3 = gp.tile([128, LC], F32, tag="g")
                    nc.vector.tensor_mul(g3, g1, g2)
                    g[1][k], g[2][k], g[3][k] = g2, g3, g4
                else:
                    for n in range(1, NST):
                        gn = gp.tile([128, LC], F32, tag="g", name=f"g{n}_{k}")
                        nc.scalar.activation(gn, dl[k], AF.Exp, scale=ACOL[(n, k)])
                        g[n][k] = gn

            # ---- du = delta * u (bf16 out) ----
            du = []
            for k in range(3):
                t = zp.tile([128, LC], BF, tag="du", name=f"du{k}")
                nc.vector.tensor_mul(t, dl[k], uk[k])
                du.append(t)

            # ---- scans ----
            hall = [[None] * 3 for _ in range(NST)]
            for n in range(NST):
                brow_a = bcd[c, 0, n: n + 1, :]
                brow_b = bcd[c, 1, n: n + 1, :]
                bb = bc.tile([128, LC], BF, tag="bb")
                nc.gpsimd.dma_start(bb, brow_a.to_broadcast([128, LC]))
                bb2 = bc.tile([128, LC], BF, tag="bb2")
                nc.gpsimd.dma_start(bb2, brow_b.to_broadcast([128, LC]))
                bbm = bc.tile([128, LC], BF, tag="bbm")
                nc.gpsimd.dma_start(bbm[0:64, :], brow_a.to_broadcast([64, LC]))
                nc.gpsimd.dma_start(bbm[64:128, :], brow_b.to_broadcast([64, LC]))
                for k, bt in enumerate((bb, bb2, bbm)):
                    dbu = wk.tile([128, LC], BF, tag="dbu", bufs=3)
                    nc.vector.tensor_mul(dbu, du[k], bt)
                    h = hp.tile([128, LC], BF, tag="h")
                    nc.vector.tensor_tensor_scan(
                        h, g[n][k], dbu, carry[n][k][:, 0:1], ALU.mult, ALU.add)
                    nc.gpsimd.tensor_copy(carry[n][k], h[:, LC - 1: LC])
                    hall[n][k] = h

            # ---- readout: z = sum_n h_n*C_n + D*u ----
            zt = []
            for k in range(3):
                wacc = None
                for n in range(NST):
                    crow_a = bcd[c, 0, NST + n: NST + n + 1, :]
                    crow_b = bcd[c, 1, NST + n: NST + n + 1, :]
                    cbt = bc.tile([128, LC], BF, tag="cb")
                    if k == 0:
                        nc.gpsimd.dma_start(cbt, crow_b.to_broadcast([128, LC]))
                    elif k == 1:
                        nc.gpsimd.dma_start(cbt, crow_a.to_broadcast([128, LC]))
                    else:
                        nc.gpsimd.dma_start(cbt[0:64, :], crow_b.to_broadcast([64, LC]))
                        nc.gpsimd.dma_start(cbt[64:128, :], crow_a.to_broadcast([64, LC]))
                    w = wk.tile([128, LC], BF, tag="wro", bufs=2)
                    nc.vector.tensor_mul(w, hall[n][k], cbt)
                    if wacc is None:
                        wacc = w
                    else:
                        w2 = wk.tile([128, LC], BF, tag="wacc", bufs=2)
                        nc.vector.tensor_add(w2, wacc, w)
                        wacc = w2
                z = zp.tile([128, LC], F32, tag="z", name=f"z{k}")
                nc.vector.scalar_tensor_tensor(
                    z, uk[k], dpk[:, k: k + 1], wacc,
                    op0=ALU.mult, op1=ALU.add)
                zt.append(z)

            # ---- transpose + layernorm + store ----
            for s in range(2):
                lo, hi = (zt[0], zt[2][0:64, :]) if s == 0 else (zt[1], zt[2][64:128, :])
                for t in range(LC // 128):
                    fs = bass.ts(t, 128)
                    ztp = tps.tile([128, D], F32, tag="ztp")
                    idh = ident[0:64, 0:64] if s == 0 else ident[64:128, 64:128]
                    nc.tensor.transpose(ztp[:, 0:128], lo[:, fs], ident)
                    nc.tensor.transpose(ztp[:, 128:D], hi[:, fs], idh)
                    stats = stp.tile([128, 6], F32, tag="stats")
                    nc.vector.bn_stats(out=stats, in_=ztp)
                    mv = stp.tile([128, 2], F32, tag="mv")
                    nc.vector.bn_aggr(out=mv, in_=stats)
                    lnv = stp.tile([128, 1], F32, tag="lnv")
                    nc.scalar.activation(lnv, mv[:, 1:2], AF.Ln, bias=epsc)
                    rstd = stp.tile([128, 1], F32, tag="rstd")
                    nc.scalar.activation(rstd, lnv, AF.Exp, scale=-0.5)
                    ot = io.tile([128, D], F32, tag="ot")
                    nc.vector.tensor_scalar(
                        out=ot, in0=ztp, scalar1=mv[:, 0:1], scalar2=rstd,
                        op0=ALU.subtract, op1=ALU.mult)
                    nc.sync.dma_start(
                        outs[s][c * LC + t * 128: c * LC + (t + 1) * 128, :], ot)

        def body(bench=False):
            if bench:
                for n in range(NST):
                    for k in range(3):
                        nc.vector.memset(carry[n][k], 0.0)
            for c in range(nchunk):
                chunk_body(c)

        if reps:
            with tc.For_i(0, reps, 1):
                body(bench=True)
        else:
            body()

    nc.compile()
    _PROGRAM_CACHE[key] = nc
    return nc


# --------------------------------------------------------------------------
# host side
# --------------------------------------------------------------------------
def _prep_core(inputs, core, length=L):
    b = core % 4
    cross = core < 4
    ax = lambda a: np.ascontiguousarray(np.asarray(a, np.float32))
    x_rgb = np.asarray(inputs["x_rgb"][b], np.float32)
    x_e = np.asarray(inputs["x_e"][b], np.float32)
    import ml_dtypes
    uta = ax(x_rgb.T)
    utc = ax(x_e.T)
    if cross:
        pta, ptb = uta, utc
        w1 = ("xp1_w", "dt1_w", "dt1_b", "A_log_1", "D_1")
        w2 = ("xp2_w", "dt2_w", "dt2_b", "A_log_2", "D_2")
    else:
        xsum = np.asarray(inputs["x_share_rgb"][b], np.float32) + \
            np.asarray(inputs["x_share_e"][b], np.float32)
        pta = ptb = ax(xsum.T)
        w1 = w2 = ("xps_w", "dts_w", "dts_b", "A_log_s", "D_s")

    def pack192(va, vb):
        out = np.empty((3, 128), np.float32)
        out[0] = va[0:128]
        out[1] = vb[0:128]
        out[2, 0:64] = va[128:192]
        out[2, 64:128] = vb[128:192]
        return out

    def side(wn):
        xp, dw, db, alog, dd = (np.asarray(inputs[k], np.float32) for k in wn)
        xpT = ax(xp.T)                      # (192, 14)
        A = -np.exp(alog)                   # (192, 4)
        return xpT, ax(dw.T), db, A, dd

    xpa, dwa, db1, A1, D1 = side(w1)
    xpb, dwb, db2, A2, D2 = side(w2)
    ascale = np.stack([pack192(A1[:, n], A2[:, n]) for n in range(NST)])
    dpkv = pack192(D1, D2)
    dtbp = pack192(db1, db2)
    int_pow = all(
        np.allclose(A[:, n], (n + 1) * A[:, 0], rtol=1e-5, atol=1e-7)
        for A in (A1, A2) for n in range(1, NST))
    return {
        "uta": uta[:, :length].astype(ml_dtypes.bfloat16),
        "utc": utc[:, :length].astype(ml_dtypes.bfloat16),
        "pta": pta[:, :length], "ptb": ptb[:, :length],
        "xpa": xpa, "xpb": xpb, "dwa": dwa, "dwb": dwb,
        "ascale": ascale, "dpk": dpkv, "dtbp": dtbp,
    }, int_pow


def kernel(**inputs):
    from concourse.bass_utils import run_bass_kernel_spmd
    maps, flags = [], []
    for core in range(8):
        m, f = _prep_core(inputs, core)
        maps.append(m)
        flags.append(f)
    nc = build_program(all(flags))
    res = run_bass_kernel_spmd(nc, maps, list(range(8))).results

    def ln_affine(y, wn, bn):
        w = np.asarray(inputs[wn], np.float32)
        bb = np.asarray(inputs[bn], np.float32)
        if np.all(w == 1.0) and np.all(bb == 0.0):
            return y
        return y * w + bb

    y1 = ln_affine(np.stack([res[b]["oa"] for b in range(4)]), "ln1_w", "ln1_b")
    y3 = ln_affine(np.stack([res[b]["ob"] for b in range(4)]), "ln2_w", "ln2_b")
    y2 = ln_affine(np.stack([res[b + 4]["oa"] for b in range(4)]), "lns_w", "lns_b")
    y4 = ln_affine(np.stack([res[b + 4]["ob"] for b in range(4)]), "lns_w", "lns_b")
    return y1, y2, y3, y4



# revision 2
# speedup vs baseline: 2.4330x; 2.4330x over previous
"""DBISSF quad selective-scan (Mamba-style) for Trainium2, 8 NeuronCores.

Problem: 4 selective scans over (B=4, L=16384, d=192, n=4) with shared/cross
projections, each followed by LayerNorm.

Sharding: 8 cores = (batch b, scan-pair). Cores 0-3 run the "cross" pair
(y_rgb, y_e) for b = core; cores 4-7 run the "shared" pair (y_sr, y_se) for
b = core-4. One SPMD program: each core computes two scans
  scan1 = (u_a, proj_A own-B/delta, cross-C from proj_B)
  scan2 = (u_c, proj_B own-B/delta, cross-C from proj_A)
Cross cores: proj_A/proj_B inputs are x_rgb/x_e themselves. Shared cores:
both proj inputs are x_share_rgb + x_share_e (projections computed twice,
identical) and C cross-read equals C_s, so the same program yields y_sr/y_se.

Device algorithm per core (channel-major scan, d=192 per scan packed with the
partner scan into 3x128-partition tile groups):
  dbl = xp_w @ p.T          (PE; moving = host-pretransposed p)
  z   = dt_w @ dbl[0:6] + dt_b  (PE, bias via augmented ones-row)
  delta = softplus(z) = ln(1+exp(z))   (ACT, exp+ln share one table set)
  dA_n = exp(A_n * delta) built as g, g^2=Sq(g), g^4=Sq(g^2), g^3=g*g^2
  dBu_n = delta*u*B_n       (B_n broadcast across partitions via DMA)
  h_n = tensor_tensor_scan(dA_n, dBu_n)  (DVE, chained across chunks)
  y  = sum_n h_n*C_n + D*u
  LN: mean/var via PE ones-matmuls on y and y^2; rstd = exp(-0.5*ln(var+eps));
  PE-transpose y to token-major; (y-mu)*rstd via tensor_scalar; DMA out.
LayerNorm affine (w,b) applied on host (w=1,b=0 in this model anyway).
"""
import sys

sys.path.insert(0, "/opt/trn_rl_repo")

import numpy as np
from contextlib import ExitStack

import concourse.bass as bass
import concourse.tile as tile
from concourse import bacc, mybir
from concourse.masks import make_identity

F32 = mybir.dt.float32
BF = mybir.dt.bfloat16
AF = mybir.ActivationFunctionType
ALU = mybir.AluOpType

# The default act-table placement maps exp->exp_and_others and
# ln->natural_log (first set containing each function), inserting a
# ~2.7us table reload before nearly every activation when Exp/Ln/Square
# interleave. All three live together in natural_log_exp_and_others, so
# restrict the placement pass to that one set: a single load at program
# start serves every activation. Ids keep their positions, so walrus
# still resolves id->name against the same act_info.json.
import concourse.bacc as _bacc_mod

_orig_gat = _bacc_mod.get_activation_tables


def _gat_one_set(arch):
    return {
        name: (fns if name == "natural_log_exp_and_others" else set())
        for name, fns in _orig_gat(arch).items()
    }


_bacc_mod.get_activation_tables = _gat_one_set

B, L, D, NST, RNK = 4, 16384, 192, 4, 6
LC = 1024               # chunk length (free dim)
NCHUNK = L // LC
TB = LC // 128          # 128-token transpose blocks per chunk
LN_EPS = 1e-5

_PROGRAM_CACHE = {}


# --------------------------------------------------------------------------
# device program
# --------------------------------------------------------------------------
def build_program(int_pow_a: bool, length=L, reps=0):
    """Build + compile the SPMD program. int_pow_a: dA powers via squaring."""
    key = (int_pow_a, length, reps)
    if key in _PROGRAM_CACHE:
        return _PROGRAM_CACHE[key]

    nchunk = length // LC
    NS = LC // 512            # 512-wide PSUM sub-slices per chunk
    nc = bacc.Bacc("TRN2", target_bir_lowering=False, debug=False, num_devices=8)

    dr = {}
    for nm in ("uta", "utc"):
        dr[nm] = nc.dram_tensor(nm, [D, length], BF, kind="ExternalInput").ap()
    for nm in ("pta", "ptb"):
        dr[nm] = nc.dram_tensor(nm, [D, length], F32, kind="ExternalInput").ap()
    for nm in ("xpa", "xpb"):
        dr[nm] = nc.dram_tensor(nm, [D, RNK + 2 * NST], F32, kind="ExternalInput").ap()
    for nm in ("dwa", "dwb"):
        dr[nm] = nc.dram_tensor(nm, [RNK, D], F32, kind="ExternalInput").ap()
    dr["dtbp"] = nc.dram_tensor("dtbp", [3, 128], F32, kind="ExternalInput").ap()
    dr["ascale"] = nc.dram_tensor("ascale", [NST, 3, 128], F32, kind="ExternalInput").ap()
    dr["dpk"] = nc.dram_tensor("dpk", [3, 128], F32, kind="ExternalInput").ap()
    bcd = nc.dram_tensor("bcd", [nchunk, 2, 2 * NST, LC], BF).ap()
    oa = nc.dram_tensor("oa", [length, D], F32, kind="ExternalOutput").ap()
    ob = nc.dram_tensor("ob", [length, D], F32, kind="ExternalOutput").ap()
    outs = (oa, ob)

    with tile.TileContext(nc) as tc, ExitStack() as ctx:
        cst = ctx.enter_context(tc.tile_pool(name="cst", bufs=1))
        io = ctx.enter_context(tc.tile_pool(name="io", bufs=2))
        wk = ctx.enter_context(tc.tile_pool(name="wk", bufs=3))
        gp = ctx.enter_context(tc.tile_pool(name="gp", bufs=13))
        bc = ctx.enter_context(tc.tile_pool(name="bc", bufs=2))
        hp = ctx.enter_context(tc.tile_pool(name="hp", bufs=14))
        zp = ctx.enter_context(tc.tile_pool(name="zpool", bufs=4))
        stp = ctx.enter_context(tc.tile_pool(name="stp", bufs=8))
        car = ctx.enter_context(tc.tile_pool(name="car", bufs=1))
        pp = ctx.enter_context(tc.tile_pool(name="pp", bufs=2, space="PSUM"))
        zps = ctx.enter_context(tc.tile_pool(name="zps", bufs=1, space="PSUM"))
        tps = ctx.enter_context(tc.tile_pool(name="tps", bufs=2, space="PSUM"))

        # ---- constants ----
        ident = cst.tile([128, 128], F32)
        make_identity(nc, ident)
        asc = cst.tile([128, NST * 3], F32)  # [p, n*3+k]
        nc.sync.dma_start(asc, dr["ascale"].rearrange("n k p -> p (n k)"))
        dpk = cst.tile([128, 3], F32)
        nc.sync.dma_start(dpk, dr["dpk"].rearrange("k p -> p k"))
        ones = cst.tile([128, 1], F32)
        nc.vector.memset(ones, 1.0)
        epsc = cst.tile([128, 1], F32)
        nc.vector.memset(epsc, LN_EPS)
        xph = cst.tile([128, RNK + 2 * NST], F32, tag="xph")
        nc.sync.dma_start(xph[0:64, :], dr["xpa"][128:D, :])
        nc.sync.dma_start(xph[64:128, :], dr["xpb"][128:D, :])
        xps = {}
        for X, hs in (("a", xph[0:64, :]), ("b", xph[64:128, :])):
            t0 = cst.tile([128, RNK + 2 * NST], F32, tag=f"xp{X}0", name=f"xp{X}0")
            nc.sync.dma_start(t0, dr[f"xp{X}"][0:128, :])
            xps[X] = (t0, hs)
        dws = {}
        for X in ("a", "b"):
            t0 = cst.tile([RNK, 128], F32, tag=f"dw{X}0", name=f"dw{X}0")
            nc.sync.dma_start(t0, dr[f"dw{X}"][:, 0:128])
            t1 = cst.tile([RNK, 64], F32, tag=f"dw{X}1", name=f"dw{X}1")
            nc.sync.dma_start(t1, dr[f"dw{X}"][:, 128:D])
            dws[X] = (t0, t1)
        dtb = cst.tile([128, 3], F32)
        nc.sync.dma_start(dtb, dr["dtbp"].rearrange("k p -> p k"))

        carry = [[car.tile([128, 1], F32, tag=f"car{n}_{k}", name=f"car{n}_{k}")
                  for k in range(3)] for n in range(NST)]
        for n in range(NST):
            for k in range(3):
                nc.vector.memset(carry[n][k], 0.0)

        ACOL = {(n, k): asc[:, n * 3 + k: n * 3 + k + 1] for n in range(NST)
                for k in range(3)}

        def chunk_body(c):
            sl = bass.ts(c, LC)
            # ---- loads (channel-major) ----
            ua = io.tile([128, LC], BF, tag="ua")
            nc.sync.dma_start(ua, dr["uta"][0:128, sl])
            ucl = io.tile([128, LC], BF, tag="uc")
            nc.sync.dma_start(ucl, dr["utc"][0:128, sl])
            um = io.tile([128, LC], BF, tag="um")
            nc.sync.dma_start(um[0:64, :], dr["uta"][128:D, sl])
            nc.sync.dma_start(um[64:128, :], dr["utc"][128:D, sl])
            pa = io.tile([128, LC], F32, tag="pa")
            nc.sync.dma_start(pa, dr["pta"][0:128, sl])
            pb = io.tile([128, LC], F32, tag="pb")
            nc.sync.dma_start(pb, dr["ptb"][0:128, sl])
            pm = io.tile([128, LC], F32, tag="pm")
            nc.sync.dma_start(pm[0:64, :], dr["pta"][128:D, sl])
            nc.sync.dma_start(pm[64:128, :], dr["ptb"][128:D, sl])
            uk = (ua, ucl, um)

            # ---- projections: dbl = xp_w @ p -> [14, LC] (512-wide slices) ----
            dbl = {}
            for X, mv_lo, mv_hi in (("a", pa, pm[0:64, :]), ("b", pb, pm[64:128, :])):
                sb = wk.tile([RNK + 2 * NST, LC], F32, tag="dbl", name=f"dbl{X}")
                for j in range(NS):
                    fs = bass.ts(j, 512)
                    ps = pp.tile([RNK + 2 * NST, 512], F32, tag="dblps")
                    nc.tensor.matmul(ps, xps[X][0], mv_lo[:, fs], start=True, stop=False)
                    nc.tensor.matmul(ps, xps[X][1], mv_hi[:, fs], start=False, stop=True)
                    nc.vector.tensor_copy(sb[:, fs], ps)
                dbl[X] = sb
                nc.gpsimd.dma_start(
                    bcd[c, 0 if X == "a" else 1], sb[RNK:RNK + 2 * NST, :])
            # ---- delta ----
            dl = []
            for k in range(3):
                d = zp.tile([128, LC], F32, tag="dl", name=f"dl{k}")
                for j in range(NS):
                    fs = bass.ts(j, 512)
                    zc = zps.tile([128, 512], F32, tag=f"zz{k}", name=f"zz{k}")
                    if k == 0:
                        nc.tensor.matmul(zc, dws["a"][0], dbl["a"][0:RNK, fs],
                                         start=True, stop=True)
                    elif k == 1:
                        nc.tensor.matmul(zc, dws["b"][0], dbl["b"][0:RNK, fs],
                                         start=True, stop=True)
                    else:
                        nc.tensor.matmul(zc[0:64, :], dws["a"][1], dbl["a"][0:RNK, fs],
                                         start=True, stop=True)
                        nc.tensor.matmul(zc[64:128, :], dws["b"][1], dbl["b"][0:RNK, fs],
                                         start=True, stop=True)
                    e = wk.tile([128, 512], F32, tag="esp", bufs=2)
                    nc.scalar.activation(e, zc, AF.Exp, bias=dtb[:, k: k + 1])
                    nc.scalar.activation(d[:, fs], e, AF.Ln, bias=1.0)
                dl.append(d)

            # ---- dA powers (bf16) ----
            g = [[None] * 3 for _ in range(NST)]
            for k in range(3):
                g1 = gp.tile([128, LC], F32, tag="g")
                nc.scalar.activation(g1, dl[k], AF.Exp, scale=ACOL[(0, k)])
                g[0][k] = g1
                if int_pow_a:
                    g2 = gp.tile([128, LC], F32, tag="g")
                    nc.scalar.activation(g2, g1, AF.Square)
                    g4 = gp.tile([128, LC], F32, tag="g")
                    nc.scalar.activation(g4, g2, AF.Square)
                    g3 = gp.tile([128, LC], F32, tag="g")
                    nc.vector.tensor_mul(g3, g1, g2)
                    g[1][k], g[2][k], g[3][k] = g2, g3, g4
                else:
                    for n in range(1, NST):
                        gn = gp.tile([128, LC], F32, tag="g", name=f"g{n}_{k}")
                        nc.scalar.activation(gn, dl[k], AF.Exp, scale=ACOL[(n, k)])
                        g[n][k] = gn

            # ---- du = delta * u (bf16 out) ----
            du = []
            for k in range(3):
                t = zp.tile([128, LC], BF, tag="du", name=f"du{k}")
                nc.vector.tensor_mul(t, dl[k], uk[k])
                du.append(t)

            # ---- scans ----
            hall = [[None] * 3 for _ in range(NST)]
            for n in range(NST):
                brow_a = bcd[c, 0, n: n + 1, :]
                brow_b = bcd[c, 1, n: n + 1, :]
                bb = bc.tile([128, LC], BF, tag="bb")
                nc.gpsimd.dma_start(bb, brow_a.to_broadcast([128, LC]))
                bb2 = bc.tile([128, LC], BF, tag="bb2")
                nc.gpsimd.dma_start(bb2, brow_b.to_broadcast([128, LC]))
                bbm = bc.tile([128, LC], BF, tag="bbm")
                nc.gpsimd.dma_start(bbm[0:64, :], brow_a.to_broadcast([64, LC]))
                nc.gpsimd.dma_start(bbm[64:128, :], brow_b.to_broadcast([64, LC]))
                for k, bt in enumerate((bb, bb2, bbm)):
                    dbu = wk.tile([128, LC], BF, tag="dbu", bufs=3)
                    nc.vector.tensor_mul(dbu, du[k], bt)
                    h = hp.tile([128, LC], BF, tag="h")
                    nc.vector.tensor_tensor_scan(
                        h, g[n][k], dbu, carry[n][k][:, 0:1], ALU.mult, ALU.add)
                    nc.gpsimd.tensor_copy(carry[n][k], h[:, LC - 1: LC])
                    hall[n][k] = h

            # ---- readout: z = sum_n h_n*C_n + D*u ----
            zt = []
            for k in range(3):
                wacc = None
                for n in range(NST):
                    crow_a = bcd[c, 0, NST + n: NST + n + 1, :]
                    crow_b = bcd[c, 1, NST + n: NST + n + 1, :]
                    cbt = bc.tile([128, LC], BF, tag="cb")
                    if k == 0:
                        nc.gpsimd.dma_start(cbt, crow_b.to_broadcast([128, LC]))
                    elif k == 1:
                        nc.gpsimd.dma_start(cbt, crow_a.to_broadcast([128, LC]))
                    else:
                        nc.gpsimd.dma_start(cbt[0:64, :], crow_b.to_broadcast([64, LC]))
                        nc.gpsimd.dma_start(cbt[64:128, :], crow_a.to_broadcast([64, LC]))
                    w = wk.tile([128, LC], BF, tag="wro", bufs=2)
                    nc.vector.tensor_mul(w, hall[n][k], cbt)
                    if wacc is None:
                        wacc = w
                    else:
                        w2 = wk.tile([128, LC], BF, tag="wacc", bufs=2)
                        nc.vector.tensor_add(w2, wacc, w)
                        wacc = w2
                z = zp.tile([128, LC], F32, tag="z", name=f"z{k}")
                nc.vector.scalar_tensor_tensor(
                    z, uk[k], dpk[:, k: k + 1], wacc,
                    op0=ALU.mult, op1=ALU.add)
                zt.append(z)

            # ---- transpose + layernorm + store ----
            for s in range(2):
                lo, hi = (zt[0], zt[2][0:64, :]) if s == 0 else (zt[1], zt[2][64:128, :])
                for t in range(LC // 128):
                    fs = bass.ts(t, 128)
                    ztp = tps.tile([128, D], F32, tag="ztp")
                    idh = ident[0:64, 0:64] if s == 0 else ident[64:128, 64:128]
                    nc.tensor.transpose(ztp[:, 0:128], lo[:, fs], ident)
                    nc.tensor.transpose(ztp[:, 128:D], hi[:, fs], idh)
                    stats = stp.tile([128, 6], F32, tag="stats")
                    nc.vector.bn_stats(out=stats, in_=ztp)
                    mv = stp.tile([128, 2], F32, tag="mv")
                    nc.vector.bn_aggr(out=mv, in_=stats)
                    lnv = stp.tile([128, 1], F32, tag="lnv")
                    nc.scalar.activation(lnv, mv[:, 1:2], AF.Ln, bias=epsc)
                    rstd = stp.tile([128, 1], F32, tag="rstd")
                    nc.scalar.activation(rstd, lnv, AF.Exp, scale=-0.5)
                    ot = io.tile([128, D], F32, tag="ot")
                    nc.vector.tensor_scalar(
                        out=ot, in0=ztp, scalar1=mv[:, 0:1], scalar2=rstd,
                        op0=ALU.subtract, op1=ALU.mult)
                    nc.sync.dma_start(
                        outs[s][c * LC + t * 128: c * LC + (t + 1) * 128, :], ot)

        def body(bench=False):
            if bench:
                for n in range(NST):
                    for k in range(3):
                        nc.vector.memset(carry[n][k], 0.0)
            for c in range(nchunk):
                chunk_body(c)

        if reps:
            with tc.For_i(0, reps, 1):
                body(bench=True)
        else:
            body()

    nc.compile()
    _PROGRAM_CACHE[key] = nc
    return nc


# --------------------------------------------------------------------------
# host side
# --------------------------------------------------------------------------
def _prep_core(inputs, core, length=L):
    b = core % 4
    cross = core < 4
    ax = lambda a: np.ascontiguousarray(np.asarray(a, np.float32))
    x_rgb = np.asarray(inputs["x_rgb"][b], np.float32)
    x_e = np.asarray(inputs["x_e"][b], np.float32)
    import ml_dtypes
    uta = ax(x_rgb.T)
    utc = ax(x_e.T)
    if cross:
        pta, ptb = uta, utc
        w1 = ("xp1_w", "dt1_w", "dt1_b", "A_log_1", "D_1")
        w2 = ("xp2_w", "dt2_w", "dt2_b", "A_log_2", "D_2")
    else:
        xsum = np.asarray(inputs["x_share_rgb"][b], np.float32) + \
            np.asarray(inputs["x_share_e"][b], np.float32)
        pta = ptb = ax(xsum.T)
        w1 = w2 = ("xps_w", "dts_w", "dts_b", "A_log_s", "D_s")

    def pack192(va, vb):
        out = np.empty((3, 128), np.float32)
        out[0] = va[0:128]
        out[1] = vb[0:128]
        out[2, 0:64] = va[128:192]
        out[2, 64:128] = vb[128:192]
        return out

    def side(wn):
        xp, dw, db, alog, dd = (np.asarray(inputs[k], np.float32) for k in wn)
        xpT = ax(xp.T)                      # (192, 14)
        A = -np.exp(alog)                   # (192, 4)
        return xpT, ax(dw.T), db, A, dd

    xpa, dwa, db1, A1, D1 = side(w1)
    xpb, dwb, db2, A2, D2 = side(w2)
    ascale = np.stack([pack192(A1[:, n], A2[:, n]) for n in range(NST)])
    dpkv = pack192(D1, D2)
    dtbp = pack192(db1, db2)
    int_pow = all(
        np.allclose(A[:, n], (n + 1) * A[:, 0], rtol=1e-5, atol=1e-7)
        for A in (A1, A2) for n in range(1, NST))
    return {
        "uta": uta[:, :length].astype(ml_dtypes.bfloat16),
        "utc": utc[:, :length].astype(ml_dtypes.bfloat16),
        "pta": pta[:, :length], "ptb": ptb[:, :length],
        "xpa": xpa, "xpb": xpb, "dwa": dwa, "dwb": dwb,
        "ascale": ascale, "dpk": dpkv, "dtbp": dtbp,
    }, int_pow


def kernel(**inputs):
    from concourse.bass_utils import run_bass_kernel_spmd
    maps, flags = [], []
    for core in range(8):
        m, f = _prep_core(inputs, core)
        maps.append(m)
        flags.append(f)
    nc = build_program(all(flags))
    res = run_bass_kernel_spmd(nc, maps, list(range(8))).results

    def ln_affine(y, wn, bn):
        w = np.asarray(inputs[wn], np.float32)
        bb = np.asarray(inputs[bn], np.float32)
        if np.all(w == 1.0) and np.all(bb == 0.0):
            return y
        return y * w + bb

    y1 = ln_affine(np.stack([res[b]["oa"] for b in range(4)]), "ln1_w", "ln1_b")
    y3 = ln_affine(np.stack([res[b]["ob"] for b in range(4)]), "ln2_w", "ln2_b")
    y2 = ln_affine(np.stack([res[b + 4]["oa"] for b in range(4)]), "lns_w", "lns_b")
    y4 = ln_affine(np.stack([res[b + 4]["ob"] for b in range(4)]), "lns_w", "lns_b")
    return y1, y2, y3, y4

